# revision 23
# baseline (speedup 1.0000x reference)
"""Bahdanau-attention scoring kernel for Trainium2 (8 NeuronCores).

reference computation:
  enc = transpose(encoderOutputs, (1,0,2))            # [B,S,H]
  energy = tanh(concat([hidden bcast, enc]) @ W^T(2H contraction) + b)
  scores = energy . v ; softmax over S -> [B,1,S]

decomposition used here:
  energy[b,s,h] = tanh( enc[b,s,:] @ W2[h,:] + ubias[b,h] )
  with W1 = W[:, :H], W2 = W[:, H:], ubias = hidden @ W1^T + b folded on the
  host (67 MFLOP, 0.05% of the 137 GFLOP total) and fed per-(b,h) as the
  ScalarE tanh activation's per-partition bias.

mixed-precision main GEMM (per-matmul PE cost is ~constant whether it's a
512-row fp16 or a 1024-row fp8 DoubleRow, so DR doubles throughput):
  - k-chunks 0..1 (256 of 1024): fp16 (near-exact), W2 slice pre-scaled by
    512 (pow2) so both paths share one PSUM scale
  - k-chunks 2..7 (768 of 1024): fp8 e4m3 (enc x16, W2 x32), DoubleRow
    matmuls over kc-pairs at 0.5 cycles/row
  => 5 matmuls per energy tile instead of 8; quantization noise on 6/8 of K
  gives rel_fro ~1.89e-2 (vs the 2e-2 gate; numpy-predicted and
  HW-reproducible since everything is deterministic).
  The x512 scale is undone by the tanh activation (tanh(psum/512 + ubias)).

sharding: data-parallel over batch B=32 -> 4 batches per core.
Per-core kernel layout:
  - energy tiles [h=128 part, rows=512 free]: 3 DR fp8 + 2 fp16 matmuls
  - tanh fused with per-partition ubias on ScalarE, output fp16
  - v-dot: one fused DVE scalar_tensor_tensor per h-chunk
    (acc = tanh*v_chunk + acc, all fp16 for the 16-bit DVE path), then one
    fp16 matmul per row-block with a one-hot ones column reduces partitions
    and lands batch bb's scores on psum partition 32*bb
  - softmax: exp of each 512-score segment straight from PSUM on ScalarE
    (accum_out collects partial sums; exp/tanh/copy share one act table so
    there are no table reloads); when a batch's 4 segments are done, a tiny
    reduce + reciprocal + two half-row scales + two half-row DMAs finish it,
    all overlapped with the next batch's compute

DMA layout: fp8 stream (w2hi + enc8 blocks) on the SP queue; fp16 stream
(w2x + enc16 blocks + consts) on the Pool queue (GpSimd COMPUTE is ~16x
slower than DVE on real TRN2 and is never used); enc prefetch runs two
blocks ahead.

toolchain notes (this container):
  - walrus here accepts only ONE sync wait per instruction; _split_multiwaits
    rewrites the BIR to single-wait NoOp chains (hooked via nc.to_json_bytes)
"""

import json
import sys
import types

import ml_dtypes
import numpy as np

H = 1024
S = 2048
B = 32
NCORES = 8
B_LOC = B // NCORES          # 4 batches per core
R = S * B_LOC                # 8192 rows per core (b-major: r = b*S + s)
NBLK = R // 512              # 16 row blocks of 512
KC = H // 128                # 8 contraction chunks
HC = H // 128                # 8 h chunks

KE = 2                       # fp16 k-chunks (kc 0..KE-1)
KQ = KC - KE                 # fp8 k-chunks (kc KE..KC-1), must be even
NP8 = KQ // 2                # DoubleRow kc-pairs

S_E = 16.0                   # enc fp8 scale
S_W = 32.0                   # W2 fp8 scale
PSC = S_E * S_W              # common psum scale (fp16 W2 slice pre-scaled)
ACT_SCALE = 1.0 / PSC

F8 = ml_dtypes.float8_e4m3


def _install_ntff_hook():
    """Install antenv.axon_hooks shim so trace=True works under axon."""
    if "antenv.axon_hooks" in sys.modules:
        return
    try:
        from trn_agent_boot.trn_boot import _ntff_profile_via_ctypes

        hook = _ntff_profile_via_ctypes("/opt/axon/libaxon_pjrt.so")
    except Exception:
        hook = None
    mod = types.ModuleType("antenv.axon_hooks")
    mod._hook = hook
    mod.get_axon_ntff_profile_hook = lambda: mod._hook

    def _set(h):
        mod._hook = h

    mod.set_axon_ntff_profile_hook = _set
    sys.modules["antenv.axon_hooks"] = mod


def _split_multiwaits(bir):
    """This walrus build supports one sync wait per instruction: split
    longer on_wait lists into single-wait NoOps on the same engine."""
    for fn in bir["functions"]:
        for blk in fn["blocks"]:
            out = []
            for inst in blk["instructions"]:
                si = inst.get("sync_info")
                ow = (si or {}).get("on_wait") or []
                if len(ow) > 1:
                    for j, w in enumerate(ow[:-1]):
                        out.append(
                            {
                                "debug": inst.get("debug", 0),
                                "engine": inst["engine"],
                                "ins": [],
                                "name": f"{inst['name']}_sw{j}",
                                "opcode": "NoOp",
                                "outs": [],
                                "sync_info": {"on_wait": [w], "on_update": []},
                                "text_hint": "waitsplit",
                            }
                        )
                    si["on_wait"] = [ow[-1]]
                out.append(inst)
            blk["instructions"] = out
    return bir


def _patch_json(nc):
    orig = nc.to_json_bytes

    def patched():
        return json.dumps(_split_multiwaits(json.loads(orig()))).encode()

    nc.to_json_bytes = patched


def build_kernel():
    import concourse.bass as bass
    import concourse.tile as tile
    from concourse import mybir

    f32 = mybir.dt.float32
    f16 = mybir.dt.float16
    f8 = mybir.dt.float8e4
    AF = mybir.ActivationFunctionType
    DR = mybir.MatmulPerfMode.DoubleRow
    MUL = mybir.AluOpType.mult
    ADD = mybir.AluOpType.add
    AX = mybir.AxisListType.X

    nc = bass.Bass("TRN2", target_bir_lowering=False, debug=False, num_devices=1)

    # All big operands are stored partition-major in DRAM ([128, chunks*cols])
    # so each SBUF tile fills with a single (or few) 3D DMA.
    enc8_t = nc.dram_tensor("enc8_t", [128, KQ * R], f8, kind="ExternalInput").ap()
    enc16_t = nc.dram_tensor("enc16_t", [128, KE * R], f16, kind="ExternalInput").ap()
    w2x = nc.dram_tensor("w2x", [128, KE * H], f16, kind="ExternalInput").ap()
    w2hi = nc.dram_tensor("w2hi", [128, KQ * H], f8, kind="ExternalInput").ap()
    ub = nc.dram_tensor("ub", [128, HC * B_LOC], f32, kind="ExternalInput").ap()
    vcol = nc.dram_tensor("vcol", [128, HC], f32, kind="ExternalInput").ap()
    onesoh = nc.dram_tensor("onesoh", [128, B_LOC * 128], f16, kind="ExternalInput").ap()
    out = nc.dram_tensor("out", [B_LOC, S], f32, kind="ExternalOutput").ap()

    # block-major layouts: each block's per-partition bytes are one
    # contiguous run, so a block DMA is 128 big descriptors, not 768
    enc8_4 = enc8_t.rearrange("p (n c r) -> p n c r", n=NBLK, c=KQ)
    enc16_4 = enc16_t.rearrange("p (n c r) -> p n c r", n=NBLK, c=KE)
    w2x3 = w2x.rearrange("p (c h) -> p c h", c=KE)
    w2hi3 = w2hi.rearrange("p (c h) -> p c h", c=KQ)

    with tile.TileContext(nc) as tc:
        with (
            tc.tile_pool(name="consts", bufs=1) as consts,
            tc.tile_pool(name="w2p", bufs=1) as w2p,
            tc.tile_pool(name="encp", bufs=3) as encp,
            tc.tile_pool(name="enc16p", bufs=3) as enc16p,
            tc.tile_pool(name="tanp", bufs=3) as tanp,
            tc.tile_pool(name="accp", bufs=2) as accp,
            tc.tile_pool(name="softp", bufs=1) as softp,
            tc.tile_pool(name="epsum", bufs=6, space="PSUM") as epsum,  # energy banks
            tc.tile_pool(name="spsum", bufs=2, space="PSUM") as spsum,  # score sums
        ):
            # ---- fp8 stream on the SP queue ------------------------------
            # fp8 pair 0 first so the pair-outer matmuls of block 0 can
            # start while the rest streams in
            w2hi_sb = w2p.tile([128, KQ, H], f8, tag="w2hi_sb")
            w2x_sb = w2p.tile([128, KE, H], f16, tag="w2x_sb")
            et8_0 = encp.tile([128, KQ, 512], f8, tag="enc8")
            et16_0 = enc16p.tile([128, KE, 512], f16, tag="enc16")
            # kc-pair-interleaved so the pair-outer matmuls of block 0
            # start as soon as the first (w2, enc) pair lands
            for p in range(NP8):
                ksl = slice(2 * p, 2 * p + 2)
                nc.sync.dma_start(w2hi_sb[:, ksl, :], w2hi3[:, ksl, :])
                nc.sync.dma_start(et8_0[:, ksl, :], enc8_4[:, 0, ksl, :])

            # ---- fp16 stream + small constants on the Pool queue ----------
            nc.gpsimd.dma_start(w2x_sb[:], w2x3[:])
            nc.gpsimd.dma_start(et16_0[:], enc16_4[:, 0])
            # tiny consts ride the idle ACT queue so ubias/vcol land first
            # and block 0's tanh can start draining PSUM banks immediately
            ubias = consts.tile([128, HC, B_LOC], f32, tag="ubias")
            nc.scalar.dma_start(ubias[:], ub.rearrange("p (c b) -> p c b", c=HC))
            vcol_sb = consts.tile([128, HC], f32, tag="vcol_sb")
            nc.scalar.dma_start(vcol_sb[:], vcol[:])

            # ones one-hot for the partition-sum matmul: column 32*bb is 1
            ones_oh = consts.tile([128, B_LOC, 128], f16, tag="ones_oh")
            nc.scalar.dma_start(
                ones_oh[:], onesoh.rearrange("p (b m) -> p b m", b=B_LOC)
            )

            # ---- softmax staging ------------------------------------------
            esc = softp.tile([128, S], f32, tag="esc")
            partial = softp.tile([128, B_LOC], f32, tag="partial")
            ssum = softp.tile([128, 1], f32, tag="ssum")
            rsum = softp.tile([128, 1], f32, tag="rsum")
            prob = softp.tile([128, S], f32, tag="prob")

            pending_sum = None  # (acc, bb, sb) awaiting partition-sum MM

            def emit_sum(pending):
                acc, bb, sb = pending
                row = slice(32 * bb, 32 * bb + 1)
                seg = slice(sb * 512, (sb + 1) * 512)
                mw = 32 * bb + 1
                sp = spsum.tile([128, 512], f32, tag="sp")
                nc.tensor.matmul(
                    sp[0:mw, :],
                    ones_oh[:, bb, 0:mw],
                    acc[:],
                    start=True,
                    stop=True,
                    skip_group_check=True,
                )
                # exp straight from PSUM; accum_out collects the segment sum
                nc.scalar.activation(
                    esc[row, seg], sp[row, :], AF.Exp, bias=0.0, scale=1.0,
                    accum_out=partial[row, sb : sb + 1],
                )
                if sb == (S // 512) - 1:
                    # batch bb complete: normalize + writeback, overlapped
                    # with the next batch's compute
                    nc.vector.tensor_reduce(
                        ssum[row, :], partial[row, :], axis=AX, op=ADD
                    )
                    nc.vector.reciprocal(rsum[row, :], ssum[row, :])
                    # split ~DVE:ACT = 1280:768 (DVE is ~1.6x faster/elem)
                    cut = 1280
                    nc.vector.tensor_scalar_mul(
                        prob[row, 0:cut], esc[row, 0:cut], rsum[row, :]
                    )
                    nc.scalar.mul(
                        prob[row, cut:S], esc[row, cut:S], rsum[row, :]
                    )
                    nc.sync.dma_start(
                        out[bb : bb + 1, 0:cut], prob[row, 0:cut]
                    )
                    nc.scalar.dma_start(
                        out[bb : bb + 1, cut:S], prob[row, cut:S]
                    )

            def emit_energy(ep, et8, et16, hsl, sl, skip_check, dr_first):
                """5 matmuls accumulating one energy psum tile. The PE pays
                ~187ns to switch INTO fp8-DoubleRow mode after an fp16
                matmul, so consecutive groups alternate their internal order
                (even: DR,DR,DR,f16,f16 / odd: f16,f16,DR,DR,DR) - the mode
                then only switches once per group pair."""
                def mm_dr(p, start, stop):
                    nc.tensor.matmul(
                        ep[:, sl],
                        w2hi_sb[:, 2 * p : 2 * p + 2, hsl],
                        et8[:, 2 * p : 2 * p + 2, sl],
                        start=start, stop=stop, perf_mode=DR,
                        skip_group_check=skip_check,
                    )
                def mm_f16(kc, start, stop):
                    nc.tensor.matmul(
                        ep[:, sl],
                        w2x_sb[:, kc, hsl],
                        et16[:, kc, sl],
                        start=start, stop=stop,
                        skip_group_check=skip_check,
                    )
                if dr_first:
                    for p in range(NP8):
                        mm_dr(p, p == 0, False)
                    for kc in range(KE):
                        mm_f16(kc, False, kc == KE - 1)
                else:
                    for kc in range(KE):
                        mm_f16(kc, kc == 0, False)
                    for p in range(NP8):
                        mm_dr(p, False, p == NP8 - 1)

            # prefetch block 1 before the main loop (2-deep from then on)
            ets = {0: (et8_0, et16_0)}

            def prefetch(nblk, q8=nc.sync, q16=nc.gpsimd):
                if nblk < NBLK and nblk not in ets:
                    et8n = encp.tile([128, KQ, 512], f8, tag="enc8")
                    et16n = enc16p.tile([128, KE, 512], f16, tag="enc16")
                    q8.dma_start(et8n[:], enc8_4[:, nblk])
                    q16.dma_start(et16n[:], enc16_4[:, nblk])
                    ets[nblk] = (et8n, et16n)

            # block 1 rides the ACT queue too: issue-parallel with the
            # sync/pool startup streams
            prefetch(1, nc.scalar, nc.scalar)
            for blk in range(NBLK):
                bb = blk // (S // 512)       # batch of this block
                sb = blk % (S // 512)        # block index within the batch
                prefetch(blk + 2)
                et8, et16 = ets.pop(blk)

                acc = accp.tile([128, 512], f16, tag="acc")

                def postproc(ep, hc):
                    # tanh with fused ubias (undoes the x512 scale), then
                    # one fused DVE op: acc = tanh*v_chunk (+ acc)
                    tt = tanp.tile([128, 512], f16, tag="tt")
                    nc.scalar.activation(
                        tt[:], ep[:], AF.Tanh,
                        bias=ubias[:, hc, bb : bb + 1], scale=ACT_SCALE,
                    )
                    if hc == 0:
                        nc.vector.tensor_scalar_mul(
                            acc[:], tt[:], vcol_sb[:, hc : hc + 1]
                        )
                    else:
                        nc.vector.scalar_tensor_tensor(
                            acc[:], tt[:], vcol_sb[:, hc : hc + 1], acc[:],
                            op0=MUL, op1=ADD,
                        )

                full = slice(0, 512)
                if blk == 0:
                    # pair-outer halves: PE can start as soon as the first
                    # (w2, enc) kc-pair lands instead of waiting for all 8.
                    # fp8 pairs first (they arrive first), fp16 last.
                    for half in range(2):
                        hcs = range(half * 4, half * 4 + 4)
                        eps = {}
                        for hc in hcs:
                            e0t = epsum.tile([128, 512], f32, tag="ep")
                            eps[hc] = e0t
                        for p in range(NP8):
                            for hc in hcs:
                                hsl = slice(hc * 128, (hc + 1) * 128)
                                nc.tensor.matmul(
                                    eps[hc][:],
                                    w2hi_sb[:, 2 * p : 2 * p + 2, hsl],
                                    et8[:, 2 * p : 2 * p + 2, :],
                                    start=(p == 0),
                                    stop=False,
                                    perf_mode=DR,
                                    skip_group_check=True,
                                )
                        for kc in range(KE):
                            for hc in hcs:
                                hsl = slice(hc * 128, (hc + 1) * 128)
                                nc.tensor.matmul(
                                    eps[hc][:],
                                    w2x_sb[:, kc, hsl],
                                    et16[:, kc, :],
                                    start=False,
                                    stop=(kc == KE - 1),
                                    skip_group_check=True,
                                )
                        for hc in hcs:
                            postproc(eps[hc], hc)
                else:
                    for hc in range(HC):
                        hsl = slice(hc * 128, (hc + 1) * 128)
                        ep = epsum.tile([128, 512], f32, tag="ep")
                        last_chunk = blk == NBLK - 1 and hc == HC - 1
                        if last_chunk:
                            # split the very last energy group into two
                            # 256-column halves so the tanh/v-dot chain (and
                            # with it the final partition-sum) starts half a
                            # group earlier - trims the end-of-stream stall
                            tt_l = tanp.tile([128, 512], f16, tag="tt")
                            for half in range(2):
                                sl = slice(half * 256, half * 256 + 256)
                                emit_energy(ep, et8, et16, hsl, sl, True, hc % 2 == 0)
                                nc.scalar.activation(
                                    tt_l[:, sl], ep[:, sl], AF.Tanh,
                                    bias=ubias[:, hc, bb : bb + 1],
                                    scale=ACT_SCALE,
                                )
                                nc.vector.scalar_tensor_tensor(
                                    acc[:, sl], tt_l[:, sl],
                                    vcol_sb[:, hc : hc + 1], acc[:, sl],
                                    op0=MUL, op1=ADD,
                                )
                            continue
                        emit_energy(ep, et8, et16, hsl, full, False, hc % 2 == 0)
                        if hc == 0 and pending_sum is not None:
                            emit_sum(pending_sum)
                            pending_sum = None
                        postproc(ep, hc)

                pending_sum = (acc, bb, sb)

            emit_sum(pending_sum)

    _patch_json(nc)
    return nc


_NC_CACHE = None


def _get_nc():
    global _NC_CACHE
    if _NC_CACHE is None:
        _NC_CACHE = build_kernel()
    return _NC_CACHE


def _part_major(x, nchunk, cols):
    """[nchunk*128, cols] row-chunked -> [128, nchunk*cols] partition-major."""
    return np.ascontiguousarray(
        x.reshape(nchunk, 128, cols).transpose(1, 0, 2).reshape(128, nchunk * cols)
    )


def shard_inputs(hidden, encoderOutputs, W, b, v):
    """Host-side prep: per-core input dict list."""
    hidden = np.ascontiguousarray(hidden, dtype=np.float32)
    W = np.ascontiguousarray(W, dtype=np.float32)
    b = np.ascontiguousarray(b, dtype=np.float32)
    v = np.ascontiguousarray(v, dtype=np.float32)

    # ubias[b, h] = hidden @ W1^T + b, folded on the host (0.05% of FLOPs)
    u_full = hidden @ W[:, :H].T + b                     # [B, H] fp32
    w2t = np.ascontiguousarray(W[:, H:].T)               # [k, h]
    w2x_pm = _part_major(
        (w2t[: KE * 128] * np.float32(PSC)).astype(np.float16), KE, H
    )
    w2hi_pm = _part_major(
        (w2t[KE * 128 :] * np.float32(S_W)).astype(F8), KQ, H
    )
    vcol = np.ascontiguousarray(v.reshape(HC, 128).T)    # [128, hc] f32
    onesoh = np.zeros((128, B_LOC, 128), np.float16)
    for bb in range(B_LOC):
        onesoh[:, bb, 32 * bb] = 1.0
    onesoh = np.ascontiguousarray(onesoh.reshape(128, B_LOC * 128))

    # [H, B, S] single big transpose, then quantize each k-range once
    encT = np.transpose(np.asarray(encoderOutputs, dtype=np.float32), (2, 1, 0))
    enc16 = encT[: KE * 128].astype(np.float16)          # [KE*128, B, S]
    enc8 = (encT[KE * 128 :] * np.float32(S_E)).astype(F8)  # [KQ*128, B, S]

    in_maps = []
    for i in range(NCORES):
        b0 = i * B_LOC
        enc8_c = np.ascontiguousarray(enc8[:, b0 : b0 + B_LOC, :]).reshape(
            KQ * 128, R
        )
        enc16_c = np.ascontiguousarray(enc16[:, b0 : b0 + B_LOC, :]).reshape(
            KE * 128, R
        )
        # ubias in [h-part 128, hc, b] layout: ub[p, hc*B_LOC + b]
        u_c = u_full[b0 : b0 + B_LOC]                    # [4, H]
        ub_pm = np.ascontiguousarray(
            u_c.T.reshape(HC, 128, B_LOC).transpose(1, 0, 2).reshape(128, HC * B_LOC)
        )
        enc8_bm = np.ascontiguousarray(
            enc8_c.reshape(KQ, 128, NBLK, 512).transpose(1, 2, 0, 3)
            .reshape(128, NBLK * KQ * 512)
        )
        enc16_bm = np.ascontiguousarray(
            enc16_c.reshape(KE, 128, NBLK, 512).transpose(1, 2, 0, 3)
            .reshape(128, NBLK * KE * 512)
        )
        in_maps.append(
            {
                "enc8_t": enc8_bm,
                "enc16_t": enc16_bm,
                "w2x": w2x_pm,
                "w2hi": w2hi_pm,
                "ub": ub_pm,
                "vcol": vcol,
                "onesoh": onesoh,
            }
        )
    return in_maps


def run(in_maps, trace=False):
    if trace:
        _install_ntff_hook()
    from concourse import bass_utils

    nc = _get_nc()
    res = bass_utils.run_bass_kernel_spmd(
        nc, in_maps, core_ids=list(range(NCORES)), trace=trace
    )
    return res


def kernel(hidden, encoderOutputs, W, b, v):
    in_maps = shard_inputs(hidden, encoderOutputs, W, b, v)
    res = run(in_maps, trace=False)
    outs = [res.results[i]["out"] for i in range(NCORES)]   # each [4, S]
    full = np.concatenate(outs, axis=0)                     # [32, S]
    return full[:, None, :].astype(np.float32)              # [32, 1, S]
